# revision 1
# baseline (speedup 1.0000x reference)
"""CBAM attention kernel v2 for Trainium2, 8-core data-parallel SPMD.

bf16 I/O halves HBM traffic vs fp32 (32 MiB -> 16 MiB per core each way);
the 2e-2 harness gate comfortably absorbs bf16 quantization (~3e-3).

Layout per core: 4 samples, each ONE SBUF tile [128, 16384] bf16 where
column = q*4096 + hw and channel c = 128*q + p (q in 0..3).

Per sample (engine balance is the point -- DVE is the scarce resource):
  - ch-avg   : ACT activation(Copy, scale=1/4096, accum_out)    4 ops
  - ch-max   : DVE tensor_scalar(mult 1.0, op1=max, accum_out)  4 ops @4x
  - q-fold   : Pool (gpsimd) tensor_max                         3 ops
  - sp-max   : PE 32 transposes of colmax -> DVE segmented reduce_max
  - sp-sum   : PE ones-matmuls -> cs32 [32,128] PSUM (j=hw-block, p2)
               -> ACT copy -> PE 2 transposes -> ACT copies -> mm0/mm1
  - 7x7 conv : 28 banded PE matmuls (bf16 bands; /512 folded in mean bands)
  - MLP      : PE fp32 matmuls; relu on DVE; sigmoid on ACT
  - b row    : PE transpose b_map -> ACT copy -> DRAM bounce ->
               broadcast-load bb [128, 4096] bf16
  - apply    : DVE tensor_scalar(a) @4x + tensor_tensor(bb) @2x, in place
"""
import sys

sys.path.insert(0, "/opt/trn_rl_repo")
import numpy as np
import ml_dtypes
import concourse.bass as bass
import concourse.bacc as bacc
import concourse.mybir as mybir
from concourse import bass_isa
from concourse import tile
from concourse.bass_utils import run_bass_kernel_spmd

ALPHA = 0.02
NCORES = 8
B, C, H, W = 32, 512, 64, 64
HW = H * W          # 4096
SPC = B // NCORES   # 4 samples per core
F32 = mybir.dt.float32
BF16 = mybir.dt.bfloat16
AF = mybir.ActivationFunctionType
ALU = mybir.AluOpType
AX = mybir.AxisListType

# conv matmul emission order: dh=0 first so the PSUM group-start write covers
# the full column range (partial-range taps accumulate afterwards)
_DH_ORDER = [0, -1, 1, -2, 2, -3, 3]


def _emit_load(nc, pools, dram, s):
    xd = dram["x"]
    xpool = pools["xq"]
    xt = xpool.tile([128, 4 * HW], BF16, tag="xt")
    for q in range(4):
        nc.sync.dma_start(xt[:, q * HW:(q + 1) * HW], xd[s, q])
    return xt


def _emit_sample(nc, pools, dram, s, xt, tc):
    xd, outd = dram["x"], dram["out"]
    w1t_t, w2t_t, bands_t, ident_t, ones_t, shift_t = (
        pools["w1t"], pools["w2t"], pools["bands"], pools["ident"],
        pools["ones"], pools["shift"])
    xpool, cmpool, jpool, spool, mpool, bpool = (
        pools["xq"], pools["colmax"], pools["junk"], pools["small"],
        pools["maps"], pools["bb"])
    aux, cspool, cpspool = pools["aux"], pools["cs"], pools["cps"]
    dpool = pools["dram"]


    stats = spool.tile([128, 8], F32, tag="stats")  # cols: avg0,max0,avg1,...
    junk_v = jpool.tile([128, HW], BF16, tag="junk_v", bufs=1)
    jv_bc = junk_v[:]

    # ---- channel stats: ch-max DVE tensor_scalar 4x; ch-sum q0 on DVE,
    #      q1-q3 on ACT (stats hold SUMS; 1/HW folded into hsum post-relu) ----
    for q in range(4):
        qs = xt[:, q * HW:(q + 1) * HW]
        nc.vector.tensor_scalar(jv_bc, qs, 1.0, None,
                                op0=ALU.mult, op1=ALU.max,
                                accum_out=stats[:, 2 * q + 1:2 * q + 2])
    if s < 2:
        for q in range(4):
            nc.vector.tensor_scalar(jv_bc, xt[:, q * HW:(q + 1) * HW], 1.0,
                                    None, op0=ALU.mult, op1=ALU.add,
                                    accum_out=stats[:, 2 * q:2 * q + 1])

    # ---- spatial max: Pool partition_all_reduce(max) per q tile ----
    ar = []
    for q in range(4):
        art = cmpool.tile([128, HW], BF16, tag="ar", bufs=4)
        nc.gpsimd.partition_all_reduce(art[:], xt[:, q * HW:(q + 1) * HW],
                                       channels=128,
                                       reduce_op=bass_isa.ReduceOp.max)
        ar.append(art)

    # ---- spatial sum: PE ones-matmuls -> cps [1,512] chunks -> c_row ----
    c_row = mpool.tile([1, HW], BF16, tag="crow", bufs=1)
    for bk in range(8):
        cps = cpspool.tile([1, 512], F32, tag="cps")
        for q in range(4):
            base = q * HW + 512 * bk
            nc.tensor.matmul(cps[:], ones_t[:, 0:1], xt[:, base:base + 512],
                             start=(q == 0), stop=(q == 3))
        nc.scalar.copy(c_row[0:1, 512 * bk:512 * bk + 512], cps[:])
    # scatter rows across partitions: ctp[p2, j] = mean sums,
    # ctp[p2, 32 + 4j + q] = per-q max rows (merged below on DVE)
    ctp = cspool.tile([128, 160], F32, tag="ctp")
    for j in range(32):
        nc.tensor.matmul(ctp[:, j:j + 1], c_row[0:1, 128 * j:128 * j + 128],
                         ones_t[0:1, 0:1])
        for q in range(4):
            nc.tensor.matmul(ctp[:, 32 + 4 * j + q:33 + 4 * j + q],
                             ar[q][0:1, 128 * j:128 * j + 128],
                             ones_t[0:1, 0:1])
    mmxf = mpool.tile([128, 64], BF16, tag="mmxf")
    nc.scalar.copy(mmxf[:, 0:32], ctp[:, 0:32])
    nc.vector.reduce_max(mmxf[:, 32:64],
                         ctp[:, 32:160].rearrange("p (j q) -> p j q", q=4),
                         axis=AX.X)
    # r=1 halves of both maps to partition base 0 via one shift matmul
    hip = aux.tile([64, 64], F32, tag="aux")
    nc.tensor.matmul(hip[:], shift_t[:, 0:64], mmxf[:])
    hi_t = mpool.tile([64, 64], BF16, tag="hit")
    nc.scalar.copy(hi_t[:], hip[:])

    # ---- 7x7 conv as banded matmuls; sigmoid into bs_map ----
    bs_map = mpool.tile([128, 32], BF16, tag="bsmap")
    for r in range(2):
        cvp = aux.tile([64, 32], F32, tag="aux")
        n_ops = 14
        idx = 0
        for dh in _DH_ORDER:
            sh = r + dh
            r_in = sh % 2
            m = (sh - r_in) // 2
            jlo = max(0, -m)
            jhi = 32 - max(0, m)
            maps = ((mmxf[0:64, 0:32], mmxf[0:64, 32:64]) if r_in == 0
                    else (hi_t[0:64, 0:32], hi_t[0:64, 32:64]))
            for mi, mp in enumerate(maps):
                lhsT = bands_t[0:64, mi * 7 + dh + 3, :]
                nc.tensor.matmul(cvp[:, jlo:jhi],
                                 lhsT, mp[0:64, jlo + m:jhi + m],
                                 start=(idx == 0), stop=(idx == n_ops - 1))
                idx += 1
        nc.scalar.activation(bs_map[64 * r:64 * r + 64, :], cvp[:], AF.Sigmoid)
    b_map = mpool.tile([128, 32], BF16, tag="bmap")
    nc.vector.tensor_scalar(b_map[:], bs_map[:], ALPHA, 1.0 - ALPHA,
                            op0=ALU.mult, op1=ALU.add)

    # ---- b -> hw-ordered row (small SBUF->SBUF DMA), then Pool broadcast ----
    bTp = aux.tile([32, 128], BF16, tag="aux")
    nc.tensor.transpose(bTp[:], b_map[:], ident_t[:])
    bTs = mpool.tile([32, 128], BF16, tag="bTs")
    nc.scalar.copy(bTs[:], bTp[:])
    if s >= 2:
        junk_a = jpool.tile([128, 512], BF16, tag="junk_a", bufs=1)
        ja_bc = (junk_a[:].rearrange("p (o b) -> p o b", o=1)
                 .broadcast_to([128, 8, 512]))
        for q in range(4):
            nc.scalar.activation(ja_bc, xt[:, q * HW:(q + 1) * HW]
                                 .rearrange("p (a b) -> p a b", b=512),
                                 AF.Copy, bias=0.0, scale=1.0,
                                 accum_out=stats[:, 2 * q:2 * q + 1])
    b_row = mpool.tile([1, HW], BF16, tag="brow", bufs=1)
    bscr = dpool.tile([1, 32, 128], BF16, tag="bscr")
    with tc.high_priority():
        nc.sync.dma_start(bscr[0, :, :], bTs[:])
        nc.sync.dma_start(b_row[0:1, :],
                          bscr[:, :, :].rearrange("o j p -> o (j p)"))
        bb = bpool.tile([128, HW], BF16, tag="bb")
        nc.gpsimd.partition_broadcast(bb[:, 0:HW // 2], b_row[0:1, 0:HW // 2])
        nc.gpsimd.partition_broadcast(bb[:, HW // 2:], b_row[0:1, HW // 2:])

    # ---- channel MLP (fp32; tiny) ----
    hps = aux.tile([32, 2], F32, tag="aux")
    for q in range(4):
        nc.tensor.matmul(hps[:], w1t_t[:, q, :], stats[:, 2 * q:2 * q + 2],
                         start=(q == 0), stop=(q == 3))
    hrelu = spool.tile([32, 2], F32, tag="hrelu")
    nc.vector.tensor_scalar_max(hrelu[:], hps[:], 0.0)
    hsum = spool.tile([32, 1], F32, tag="hsum")
    nc.vector.scalar_tensor_tensor(hsum[:], hrelu[:, 0:1], 1.0 / HW,
                                   hrelu[:, 1:2], op0=ALU.mult, op1=ALU.add)
    mcps = aux.tile([128, 4], F32, tag="aux")
    for q in range(4):
        nc.tensor.matmul(mcps[:, q:q + 1], w2t_t[:, 128 * q:128 * q + 128],
                         hsum[:])
    sg = spool.tile([128, 4], F32, tag="sg")
    nc.scalar.activation(sg[:], mcps[:], AF.Sigmoid)
    a_col = spool.tile([128, 4], F32, tag="acol")
    nc.vector.tensor_scalar(a_col[:], sg[:], ALPHA, 1.0 - ALPHA,
                            op0=ALU.mult, op1=ALU.add)

    # ---- apply attention on DVE: x *= a[c] (TS 4x), x *= b[hw] (TT 2x) ----
    for q in range(4):
        qs = xt[:, q * HW:(q + 1) * HW]
        nc.vector.tensor_scalar_mul(qs, qs, a_col[:, q:q + 1])
        nc.vector.tensor_mul(qs[:, 0:HW // 2], qs[:, 0:HW // 2],
                             bb[:, 0:HW // 2])
        nc.vector.tensor_mul(qs[:, HW // 2:], qs[:, HW // 2:],
                             bb[:, HW // 2:])
        nc.sync.dma_start(outd[s, q], qs)



def build_nc(spc=SPC):
    nc = bacc.Bacc("TRN2", target_bir_lowering=False, debug=False)
    dram = {
        "x": nc.declare_dram_parameter("x", [spc, 4, 128, HW], BF16,
                                       isOutput=False),
        "w1t": nc.declare_dram_parameter("w1t", [128, 4, 32], F32,
                                         isOutput=False),
        "w2t": nc.declare_dram_parameter("w2t", [32, 512], F32, isOutput=False),
        "bands": nc.declare_dram_parameter("bands", [128, 14, 64], BF16,
                                           isOutput=False),
        "ident": nc.declare_dram_parameter("ident", [128, 128], BF16,
                                           isOutput=False),
        "ones": nc.declare_dram_parameter("ones", [128, 1], BF16,
                                          isOutput=False),
        "shift": nc.declare_dram_parameter("shift", [128, 64], BF16,
                                           isOutput=False),
        "out": nc.declare_dram_parameter("out", [spc, 4, 128, HW], BF16,
                                         isOutput=True),
    }
    with tile.TileContext(nc) as tc:
        with (
            tc.tile_pool(name="const", bufs=1) as cpool,
            tc.tile_pool(name="xq", bufs=4) as xpool,
            tc.tile_pool(name="colmax", bufs=2) as cmpool,
            tc.tile_pool(name="junk", bufs=2) as jpool,
            tc.tile_pool(name="small", bufs=2) as spool,
            tc.tile_pool(name="maps", bufs=2) as mpool,
            tc.tile_pool(name="bb", bufs=2) as bpool,
            tc.tile_pool(name="dram", bufs=2, space="DRAM") as dpool,
            tc.tile_pool(name="aux", bufs=2, space="PSUM") as aux,
            tc.tile_pool(name="cs", bufs=2, space="PSUM") as cspool,
            tc.tile_pool(name="cps", bufs=4, space="PSUM") as cpspool,
        ):
            pools = {
                "xq": xpool, "colmax": cmpool, "junk": jpool, "small": spool,
                "maps": mpool, "bb": bpool,
                "aux": aux, "cs": cspool, "cps": cpspool, "dram": dpool,
                "w1t": cpool.tile([128, 4, 32], F32, tag="w1t", name="w1t_sb"),
                "w2t": cpool.tile([32, 512], F32, tag="w2t", name="w2t_sb"),
                "bands": cpool.tile([128, 14, 64], BF16, tag="bands",
                                    name="bands_sb"),
                "ident": cpool.tile([128, 128], BF16, tag="ident",
                                    name="ident_sb"),
                "ones": cpool.tile([128, 1], BF16, tag="ones", name="ones_sb"),
                "shift": cpool.tile([128, 64], BF16, tag="shift",
                                    name="shift_sb"),
            }
            for name in ("w1t", "w2t", "bands", "ident", "ones", "shift"):
                nc.sync.dma_start(pools[name][:], dram[name][:])
            warm = cpool.tile([1, 1], F32, tag="warm", name="warm_sb")
            nc.scalar.activation(warm[:], pools["ones"][0:1, 0:1], AF.Sigmoid)
            xts = [_emit_load(nc, pools, dram, s) for s in range(spc)]
            for s in range(spc):
                _emit_sample(nc, pools, dram, s, xts[s], tc)
    nc.compile()
    return nc


def make_consts(w1, w2, wconv):
    w1t = np.ascontiguousarray(
        w1.T.reshape(4, 128, 32).transpose(1, 0, 2)).astype(np.float32)
    w2t = np.ascontiguousarray(w2.T).astype(np.float32)
    bands = np.zeros((2, 7, 64, 64), np.float32)
    for ci in range(2):
        k = wconv[0, ci]
        for dh in range(7):
            for dw in range(7):
                diag = dw - 3  # w_in - w_out
                v = np.float32(k[dh, dw])
                idx = np.arange(max(0, -diag), min(64, 64 - diag))
                bands[ci, dh, idx + diag, idx] = v
    bands[0] /= 512.0
    bands_r = np.ascontiguousarray(
        bands.transpose(2, 0, 1, 3).reshape(64, 14, 64))
    bands_r = np.ascontiguousarray(np.concatenate([bands_r, bands_r], axis=0))
    ident = np.eye(128)
    ones = np.ones((128, 1))
    shift = np.zeros((128, 64))
    shift[np.arange(64) + 64, np.arange(64)] = 1.0
    return {"w1t": w1t, "w2t": w2t,
            "bands": bands_r.astype(ml_dtypes.bfloat16),
            "ident": ident.astype(ml_dtypes.bfloat16),
            "ones": ones.astype(ml_dtypes.bfloat16),
            "shift": shift.astype(ml_dtypes.bfloat16)}


_NC = None


def kernel(**inputs):
    global _NC
    x = np.asarray(inputs["x"], dtype=np.float32)
    w1 = np.asarray(inputs["w1"], dtype=np.float32)
    w2 = np.asarray(inputs["w2"], dtype=np.float32)
    wconv = np.asarray(inputs["wconv"], dtype=np.float32)

    if _NC is None:
        _NC = build_nc()
    consts = make_consts(w1, w2, wconv)
    xb = np.ascontiguousarray(x).astype(ml_dtypes.bfloat16)
    shards = xb.reshape(NCORES, SPC, 4, 128, HW)
    in_maps = [dict(consts, x=np.ascontiguousarray(shards[i]))
               for i in range(NCORES)]
    res = run_bass_kernel_spmd(_NC, in_maps, core_ids=list(range(NCORES)))
    out = np.concatenate(
        [np.asarray(res.results[i]["out"]).astype(np.float32)
         .reshape(SPC, C, H, W) for i in range(NCORES)], axis=0)
    return out

